# revision 37
# baseline (speedup 1.0000x reference)
"""Trainium2 Bass kernel for nn_ActionThenNodePolicy (segment softmax policy head).

kernel(**inputs) takes FULL inputs (as from setup_inputs()) and returns the full
(logprob, entropy, value, p_a, p_n__a) tuple, computing the heavy O(N*D)/O(N*A)
work on 8 NeuronCores via bass/Tile.

Self-contained: hardcodes N=131072, G=256, D=256, A=64, 8 cores.
"""
import sys
import types
import numpy as np

# ---------------------------------------------------------------------------
# constants
N, G, D, A = 131072, 256, 256, 64
NCORES = 8
GPC = G // NCORES          # graphs per core
NEG = -1e9
EPS = 1e-10
TILE = 128                 # nodes per tile
SUB = 3                    # tiles per supertile (psum-bank limited)
STN = TILE * SUB           # nodes per supertile
JC = 258                   # proj output cols: agn(64) nga(64) qna(64) qan(64) node(1) pad(1)
SCOLS = 384                # contrib(64) tEL(64) tEQ(64) e_nga(64) qna(64) qan(64)
BCHUNK = 8                 # graphs per phase-B batch (also seg psum slot rows)


def _register_ntff_hook():
    try:
        from trn_agent_boot.trn_boot import _ntff_profile_via_ctypes
        if 'antenv.axon_hooks' not in sys.modules:
            hook = _ntff_profile_via_ctypes('/opt/axon/libaxon_pjrt.so')
            mod = types.ModuleType('antenv.axon_hooks')
            mod.get_axon_ntff_profile_hook = lambda: hook
            sys.modules['antenv.axon_hooks'] = mod
    except Exception:
        pass


# ---------------------------------------------------------------------------
# layout computed from runtime `indices` (sorted graph ids)

class Layout:
    def __init__(self, indices):
        counts = np.bincount(np.asarray(indices, np.int64), minlength=G)
        self.counts = counts                       # n_g actual per graph
        Lg = ((counts + TILE - 1) // TILE) * TILE  # padded length per graph
        self.Lg = Lg
        per_core = Lg.reshape(NCORES, GPC).sum(1)
        ns = int(per_core.max())
        ns = ((ns + STN - 1) // STN) * STN
        self.NS = ns
        self.T = ns // TILE
        self.ST = ns // STN
        # per-core per-graph offsets in padded space
        self.offs = np.zeros((NCORES, GPC), np.int64)
        for c in range(NCORES):
            o = 0
            for j in range(GPC):
                self.offs[c, j] = o
                o += Lg[GPC * c + j]
        self.used = per_core                        # padded nodes actually used per core
        # graph start offsets in the ORIGINAL node array
        self.gstart = np.zeros(G + 1, np.int64)
        self.gstart[1:] = np.cumsum(counts)
        # per-core tile -> local graph (or -1 for junk tail tiles); identical
        # tile->graph maps are required across cores for a single SPMD program,
        # so the program bakes PER-CORE structure only if maps match; otherwise
        # we bake the union (core-specific seg matmul emission is impossible in
        # SPMD).  We therefore make the PROGRAM data-driven per tile only via
        # input tensors (segw), but seg-matmul grouping (which psum slot, which
        # start/stop) must be identical across cores -> we pad every core to
        # the SAME per-graph tile counts: use max tiles per graph-slot across
        # cores.
        TgSlot = (Lg.reshape(NCORES, GPC) // TILE)
        self.slot_tiles = TgSlot.max(0)             # tiles per local-graph slot, shared
        ns2 = int(self.slot_tiles.sum()) * TILE
        ns2 = ((ns2 + STN - 1) // STN) * STN
        self.NS = ns2
        self.T = ns2 // TILE
        self.ST = ns2 // STN
        self.offs = np.zeros(GPC, np.int64)
        o = 0
        for j in range(GPC):
            self.offs[j] = o
            o += self.slot_tiles[j] * TILE
        self.used_slots = o // TILE                 # non-junk tiles
        # tile -> local graph slot (-1 junk)
        t2g = np.full(self.T, -1, np.int64)
        for j in range(GPC):
            t0 = self.offs[j] // TILE
            t2g[t0:t0 + self.slot_tiles[j]] = j
        self.t2g = t2g

    def key(self):
        return (self.NS, tuple(self.slot_tiles.tolist()))


# ---------------------------------------------------------------------------
# program builder (baked on layout.key())

_PROG_CACHE = {}


def _build_program(lay: Layout):
    import concourse.bass as bass
    import concourse.tile as tile
    from concourse import bacc, mybir

    F32 = mybir.dt.float32
    F32R = mybir.dt.float32r
    AF = mybir.ActivationFunctionType
    OP = mybir.AluOpType
    NSL, T, ST = lay.NS, lay.T, lay.ST
    t2g = lay.t2g

    nc = bacc.Bacc("TRN2", target_bir_lowering=False, debug=False)

    xt_d = nc.dram_tensor("xt", [128, ST, 2, SUB, 128], F32R, kind="ExternalInput").ap()
    w_d = nc.dram_tensor("w", [128, 2, JC], F32R, kind="ExternalInput").ap()
    segw_d = nc.dram_tensor("segw", [128, T, BCHUNK], F32R, kind="ExternalInput").ap()
    maskpt_d = nc.dram_tensor("maskpt", [128, T, A], F32, kind="ExternalInput").ap()
    rsd_d = nc.dram_tensor("rsd", [(GPC + BCHUNK - 1) // BCHUNK, BCHUNK * A], F32).ap()

    pna_d = nc.dram_tensor("pna", [T, 128, A], F32, kind="ExternalOutput").ap()
    pa_d = nc.dram_tensor("pa", [GPC, A], F32, kind="ExternalOutput").ap()
    h2_d = nc.dram_tensor("h2", [GPC, A], F32, kind="ExternalOutput").ap()
    s2_d = nc.dram_tensor("s2", [GPC, A], F32, kind="ExternalOutput").ap()
    qa_d = nc.dram_tensor("qa", [GPC, A], F32, kind="ExternalOutput").ap()
    sn_d = nc.dram_tensor("sn", [GPC, A], F32, kind="ExternalOutput").ap()

    with tile.TileContext(nc) as tc:
        from contextlib import ExitStack
        with ExitStack() as ctx:
            const = ctx.enter_context(tc.tile_pool(name="const", bufs=1))
            xtp = ctx.enter_context(tc.tile_pool(name="xtp", bufs=6))
            projp = ctx.enter_context(tc.tile_pool(name="projp", bufs=2, space="PSUM"))
            segp = ctx.enter_context(tc.tile_pool(name="segp", bufs=2, space="PSUM"))
            sp = ctx.enter_context(tc.tile_pool(name="sp", bufs=4))
            smallp = ctx.enter_context(tc.tile_pool(name="smallp", bufs=6))
            rsbp = ctx.enter_context(tc.tile_pool(name="rsbp", bufs=2))
            outp = ctx.enter_context(tc.tile_pool(name="outp", bufs=4))
            bwork = ctx.enter_context(tc.tile_pool(name="bwork", bufs=2))

            # ---- constants / inputs resident in SBUF ----
            wsb = const.tile([128, 2, JC], F32R)
            nc.sync.dma_start(wsb[:], w_d)
            segwsb = const.tile([128, T, BCHUNK], F32R)
            nc.sync.dma_start(segwsb[:], segw_d)
            maskpt = const.tile([128, T, A], F32)
            nc.sync.dma_start(maskpt[:], maskpt_d)
            # e_nga/e_agn cache for the whole core
            cache = const.tile([128, T, 130], F32)
            seg_slots = {}
            rsb_tiles = {}
            tbl_tiles = {}
            tblp = ctx.enter_context(tc.tile_pool(name="tblp", bufs=3))

            # ---------------- phase A ----------------
            def emit_A(st):
                xtt = xtp.tile([128, 2, SUB, 128], F32R, tag="xt")
                nc.sync.dma_start(xtt[:], xt_d[:, st, :, :, :])
                pj = projp.tile([128, SUB, 512], F32, tag="proj")
                for q in range(SUB):
                    for k in range(2):
                        nc.tensor.matmul(pj[:, q, 0:JC], xtt[:, k, q, :],
                                         wsb[:, k, :], start=(k == 0), stop=(k == 1))
                # exp of [agn|nga|node|pad] region into cache (one op)
                nc.scalar.activation(cache[:, st * SUB:(st + 1) * SUB, :],
                                     pj[:, :, 0:130], AF.Exp)
                s_t = sp.tile([128, SUB, SCOLS], F32R, tag="stage")
                # bulk-copy qna|qan into staging: S cols 256:384
                nc.scalar.copy(s_t[:, :, 256:384], pj[:, :, 130:258])
                nc.scalar.copy(s_t[:, :, 192:256],
                               cache[:, st * SUB:(st + 1) * SUB, 64:128])
                em = smallp.tile([128, SUB, A], F32, tag="em")
                nc.vector.tensor_mul(em[:], cache[:, st * SUB:(st + 1) * SUB, 0:64],
                                     maskpt[:, st * SUB:(st + 1) * SUB, :])
                den = smallp.tile([128, SUB], F32, tag="den")
                nc.vector.reduce_sum(den[:], em[:], axis=mybir.AxisListType.X)
                recip = smallp.tile([128, SUB], F32, tag="recip")
                nc.vector.reciprocal(recip[:], den[:])
                u = smallp.tile([128, SUB], F32, tag="u")
                nc.vector.tensor_mul(u[:], cache[:, st * SUB:(st + 1) * SUB, 128],
                                     recip[:])
                # contrib = em * u, one op (u broadcast along innermost free dim)
                ub = bass.AP(tensor=u.tensor, offset=u.offset,
                             ap=[u.ap[0], u.ap[1], [0, A]])
                nc.vector.tensor_mul(s_t[:, :, 0:64], em[:], ub)
                # t_EL = e_nga * nga_logit ; t_EQ = e_nga * qna (SBUF)
                nc.vector.tensor_mul(
                    s_t[:, :, 64:128],
                    cache[:, st * SUB:(st + 1) * SUB, 64:128], pj[:, :, 64:128])
                nc.vector.tensor_mul(
                    s_t[:, :, 128:192],
                    cache[:, st * SUB:(st + 1) * SUB, 64:128],
                    s_t[:, :, 256:320].bitcast(F32))
                # seg matmuls (one [BCHUNK,320] psum slot per graph batch)
                for q in range(SUB):
                    t = st * SUB + q
                    g = int(t2g[t])
                    if g < 0:
                        continue
                    b = g // BCHUNK
                    bt0 = lay.offs[b * BCHUNK] // TILE
                    glast = min(b * BCHUNK + BCHUNK, GPC) - 1
                    bt1 = lay.offs[glast] // TILE + lay.slot_tiles[glast]
                    if t == bt0:
                        seg_slots[b] = segp.tile([BCHUNK, 512], F32, tag="seg",
                                                 name=f"segslot{b}")
                    nc.tensor.matmul(seg_slots[b][:, 0:SCOLS], segwsb[:, t, :],
                                     s_t[:, q, :], start=(t == bt0),
                                     stop=(t == bt1 - 1))
                    if t == bt1 - 1:
                        tbl_tiles[b] = tblp.tile([BCHUNK, SCOLS], F32, tag="tbl",
                                                 name=f"tbl{b}")
                        nc.scalar.copy(tbl_tiles[b][:],
                                       seg_slots[b][:, 0:SCOLS])

            # ---------------- phase B (batch of graphs) ----------------
            def emit_B(g0, g1):
                n = g1 - g0
                tbl = tbl_tiles[g0 // BCHUNK]
                rs_t = bwork.tile([BCHUNK, A], F32, tag="rs")
                b = g0 // BCHUNK
                nc.vector.reciprocal(rs_t[:n, :], tbl[:n, 192:256])
                nc.sync.dma_start(rsd_d[b:b + 1, 0:n * A], rs_t[:n, :])
                rsb_tiles[b] = rsbp.tile([128, BCHUNK * A], F32, tag="rsb",
                                         name=f"rsb{b}")
                row = rsd_d[b:b + 1, 0:n * A]
                bcast = bass.AP(tensor=row.tensor, offset=row.offset,
                                ap=[[0, 128]] + list(row.ap)[1:])
                nc.sync.dma_start(rsb_tiles[b][:, 0:n * A], bcast)
                # raw per-graph blocks out; host finalizes H2/S2/p_a/q_a
                nc.sync.dma_start(pa_d[g0:g1, :], tbl[:n, 0:64])
                nc.sync.dma_start(h2_d[g0:g1, :], tbl[:n, 64:128])
                nc.sync.dma_start(s2_d[g0:g1, :], tbl[:n, 128:192])
                nc.sync.dma_start(sn_d[g0:g1, :], tbl[:n, 192:256])
                nc.sync.dma_start(qa_d[g0:g1, :], tbl[:n, 320:384])

            # ---------------- phase C (per graph) ----------------
            def emit_C(g):
                nt = int(lay.slot_tiles[g])
                if nt == 0:
                    return
                t0 = lay.offs[g] // TILE
                rsb = rsb_tiles[g // BCHUNK]
                j = g % BCHUNK
                out_t = outp.tile([128, nt, A], F32, tag="pnaout")
                base = rsb[:, j * A:(j + 1) * A]
                in2 = bass.AP(tensor=base.tensor, offset=base.offset,
                              ap=[base.ap[0], [0, nt], base.ap[1]])
                nc.vector.tensor_mul(out_t[:, 0:nt, :],
                                     cache[:, t0:t0 + nt, 64:128], in2)
                nc.sync.dma_start(pna_d[t0:t0 + nt, :, :].rearrange("t p a -> p t a"),
                                  out_t[:, 0:nt, :])

            # ---------------- emission ----------------
            import concourse.mybir as mybir  # noqa: F811
            graph_last_st = {}
            for g in range(GPC):
                if lay.slot_tiles[g] > 0:
                    tlast = lay.offs[g] // TILE + lay.slot_tiles[g] - 1
                    graph_last_st[g] = tlast // SUB
            done_b = 0
            pending_c = []
            for st in range(ST):
                emit_A(st)
                # B for any batch fully closed
                while done_b + BCHUNK <= GPC and all(
                        graph_last_st.get(g, -1) <= st
                        for g in range(done_b, done_b + BCHUNK)):
                    g0 = done_b
                    emit_B(g0, g0 + BCHUNK)
                    for g in range(g0, g0 + BCHUNK):
                        emit_C(g)
                    done_b += BCHUNK
            while done_b < GPC:
                g0 = done_b
                emit_B(g0, min(g0 + BCHUNK, GPC))
                for g in range(g0, min(g0 + BCHUNK, GPC)):
                    emit_C(g)
                done_b += BCHUNK

    nc.compile()
    return nc


# ---------------------------------------------------------------------------
# host marshaling + execution

def _prep_core_inputs(lay: Layout, c, values, action_mask, b_agn, W_all, segw_all):
    NSL, T, ST = lay.NS, lay.T, lay.ST
    xt = np.zeros((NSL, D), np.float32)
    for j in range(GPC):
        g = GPC * c + j
        n0, n1 = lay.gstart[g], lay.gstart[g + 1]
        o = lay.offs[j]
        xt[o:o + (n1 - n0)] = values[n0:n1]
    # [p, st, k, q, n] partition-major
    xtT = np.ascontiguousarray(
        xt.reshape(ST, SUB, 128, 2, 128).transpose(4, 0, 3, 1, 2))
    mrows = (action_mask[GPC * c:GPC * (c + 1)].astype(np.float32)
             * np.exp(b_agn)[None, :])                 # [GPC, A]
    mpt = mrows[np.maximum(lay.t2g, 0)]                # [T, A]
    maskpt = np.broadcast_to(mpt.reshape(-1)[None, :],
                             (128, T * A)).reshape(128, T, A).copy()
    return {"xt": xtT, "w": W_all, "segw": segw_all, "maskpt": maskpt}


def kernel(values, indices, a_action, a_node, action_mask, n_nodes,
           w_node, W_agn, b_agn, W_nga, b_nga, W_qna, b_qna, W_qan, b_qan):
    _register_ntff_hook()
    from concourse.bass_utils import run_bass_kernel_spmd

    values = np.asarray(values, np.float32)
    indices = np.asarray(indices, np.int64)
    a_action = np.asarray(a_action, np.int64)
    a_node = np.asarray(a_node, np.int64)
    action_mask = np.asarray(action_mask, bool)
    n_nodes = np.asarray(n_nodes, np.int64)
    w_node = np.asarray(w_node, np.float32)
    b_agn = np.asarray(b_agn, np.float32)
    b_nga = np.asarray(b_nga, np.float32)
    b_qna = np.asarray(b_qna, np.float32)
    b_qan = np.asarray(b_qan, np.float32)

    lay = Layout(indices)
    key = lay.key()
    if key not in _PROG_CACHE:
        _PROG_CACHE[key] = _build_program(lay)
    nc = _PROG_CACHE[key]

    # weights: cols [agn | nga | qna | qan | node | 0]
    W_all = np.zeros((D, JC), np.float32)
    W_all[:, 0:64] = np.asarray(W_agn, np.float32).T
    W_all[:, 64:128] = np.asarray(W_nga, np.float32).T
    W_all[:, 128] = w_node
    W_all[:, 130:194] = np.asarray(W_qna, np.float32).T
    W_all[:, 194:258] = np.asarray(W_qan, np.float32).T
    W_all = np.ascontiguousarray(W_all.reshape(2, 128, JC).transpose(1, 0, 2))

    # per-core segw is CORE-SPECIFIC (valid counts per graph differ)
    counts = lay.counts.reshape(NCORES, GPC)
    in_maps = []
    for c in range(NCORES):
        segw = np.zeros((lay.T, 128, BCHUNK), np.float32)
        for j in range(GPC):
            t0 = lay.offs[j] // TILE
            nv = int(counts[c, j])
            col = j % BCHUNK
            full, rem = nv // TILE, nv % TILE
            segw[t0:t0 + full, :, col] = 1.0
            if rem:
                segw[t0 + full, 0:rem, col] = 1.0
        segw = np.ascontiguousarray(segw.transpose(1, 0, 2))
        in_maps.append(_prep_core_inputs(lay, c, values, action_mask, b_agn,
                                         W_all, segw))

    import os
    if os.environ.get("KSIM") == "1":
        from concourse.bass_interp import CoreSim
        sim = CoreSim(nc)
        for name, arr in in_maps[0].items():
            sim.tensor(name)[:] = arr
        sim.simulate()
        r0 = {name: np.array(sim.tensor(name))
              for name in ["pna", "pa", "h2", "s2", "qa", "sn"]}
        zeros = {k: np.zeros_like(v) for k, v in r0.items()}

        class _R:
            results = [r0] + [zeros] * (NCORES - 1)
            exec_time_ns = None
        res = _R()
    else:
        res = run_bass_kernel_spmd(nc, in_maps, list(range(NCORES)))
    kernel.last_results = res

    # ---------------- host assembly ----------------
    pa_raw = np.zeros((G, A), np.float32)
    segEL = np.zeros((G, A), np.float32)
    segEQ = np.zeros((G, A), np.float32)
    s_nga = np.zeros((G, A), np.float32)
    q_a = np.zeros((G, A), np.float32)
    p_n__a = np.zeros((N, A), np.float32)
    for c in range(NCORES):
        r = res.results[c]
        pa_raw[GPC * c:GPC * (c + 1)] = r["pa"]
        segEL[GPC * c:GPC * (c + 1)] = r["h2"]
        segEQ[GPC * c:GPC * (c + 1)] = r["s2"]
        s_nga[GPC * c:GPC * (c + 1)] = r["sn"]
        q_a[GPC * c:GPC * (c + 1)] = r["qa"]
        pna = r["pna"].reshape(lay.NS, A)
        for j in range(GPC):
            g = GPC * c + j
            n0, n1 = lay.gstart[g], lay.gstart[g + 1]
            o = lay.offs[j]
            p_n__a[n0:n1] = pna[o:o + (n1 - n0)]

    # per-graph [G, A] finalization on host (tiny)
    rs = 1.0 / s_nga
    p_a = pa_raw / pa_raw.sum(axis=-1, keepdims=True)
    H2 = np.log(s_nga) - rs * segEL
    S2 = rs * segEQ
    # bias corrections (biases are zero in the reference setup, but be exact)
    ng = n_nodes.astype(np.float32)
    q_a = q_a + ng[:, None] * b_qan[None, :]
    S2 = S2 + b_qna[None, :]

    # entropy
    H_a = -np.sum(p_a * np.log(p_a + EPS), axis=-1)
    entropy = H_a + np.sum(p_a * H2, axis=-1)
    # value
    value = np.sum(p_a * (q_a + S2), axis=-1)
    # logprob
    g_ar = np.arange(G)
    pa_sel = p_a[g_ar, a_action]
    gprime = indices[a_node]                      # graph of the selected node
    mask_sel = action_mask[gprime, a_action]
    pnam_sel = np.where(mask_sel,
                        p_n__a[a_node, a_action],
                        1.0 / n_nodes[gprime].astype(np.float32))
    logprob = np.log(pa_sel + EPS) + np.log(pnam_sel + EPS)

    return (logprob.astype(np.float32), entropy.astype(np.float32),
            value.astype(np.float32), p_a, p_n__a)


# revision 38
# speedup vs baseline: 1.1026x; 1.1026x over previous
"""Trainium2 Bass kernel for nn_ActionThenNodePolicy (segment softmax policy head).

kernel(**inputs) takes FULL inputs (as from setup_inputs()) and returns the full
(logprob, entropy, value, p_a, p_n__a) tuple, computing the heavy O(N*D)/O(N*A)
work on 8 NeuronCores via bass/Tile.

Self-contained: hardcodes N=131072, G=256, D=256, A=64, 8 cores.
"""
import sys
import types
import numpy as np

# ---------------------------------------------------------------------------
# constants
N, G, D, A = 131072, 256, 256, 64
NCORES = 8
GPC = G // NCORES          # graphs per core
NEG = -1e9
EPS = 1e-10
TILE = 128                 # nodes per tile
SUB = 3                    # tiles per supertile (psum-bank limited)
STN = TILE * SUB           # nodes per supertile
JC = 258                   # proj output cols: agn(64) nga(64) qna(64) qan(64) node(1) pad(1)
SCOLS = 384                # contrib(64) tEL(64) tEQ(64) e_nga(64) qna(64) qan(64)
BCHUNK = 8                 # graphs per phase-B batch (also seg psum slot rows)


def _register_ntff_hook():
    try:
        from trn_agent_boot.trn_boot import _ntff_profile_via_ctypes
        if 'antenv.axon_hooks' not in sys.modules:
            hook = _ntff_profile_via_ctypes('/opt/axon/libaxon_pjrt.so')
            mod = types.ModuleType('antenv.axon_hooks')
            mod.get_axon_ntff_profile_hook = lambda: hook
            sys.modules['antenv.axon_hooks'] = mod
    except Exception:
        pass


# ---------------------------------------------------------------------------
# layout computed from runtime `indices` (sorted graph ids)

class Layout:
    def __init__(self, indices):
        counts = np.bincount(np.asarray(indices, np.int64), minlength=G)
        self.counts = counts                       # n_g actual per graph
        Lg = ((counts + TILE - 1) // TILE) * TILE  # padded length per graph
        self.Lg = Lg
        per_core = Lg.reshape(NCORES, GPC).sum(1)
        ns = int(per_core.max())
        ns = ((ns + STN - 1) // STN) * STN
        self.NS = ns
        self.T = ns // TILE
        self.ST = ns // STN
        # per-core per-graph offsets in padded space
        self.offs = np.zeros((NCORES, GPC), np.int64)
        for c in range(NCORES):
            o = 0
            for j in range(GPC):
                self.offs[c, j] = o
                o += Lg[GPC * c + j]
        self.used = per_core                        # padded nodes actually used per core
        # graph start offsets in the ORIGINAL node array
        self.gstart = np.zeros(G + 1, np.int64)
        self.gstart[1:] = np.cumsum(counts)
        # per-core tile -> local graph (or -1 for junk tail tiles); identical
        # tile->graph maps are required across cores for a single SPMD program,
        # so the program bakes PER-CORE structure only if maps match; otherwise
        # we bake the union (core-specific seg matmul emission is impossible in
        # SPMD).  We therefore make the PROGRAM data-driven per tile only via
        # input tensors (segw), but seg-matmul grouping (which psum slot, which
        # start/stop) must be identical across cores -> we pad every core to
        # the SAME per-graph tile counts: use max tiles per graph-slot across
        # cores.
        TgSlot = (Lg.reshape(NCORES, GPC) // TILE)
        self.slot_tiles = TgSlot.max(0)             # tiles per local-graph slot, shared
        ns2 = int(self.slot_tiles.sum()) * TILE
        ns2 = ((ns2 + STN - 1) // STN) * STN
        self.NS = ns2
        self.T = ns2 // TILE
        self.ST = ns2 // STN
        self.offs = np.zeros(GPC, np.int64)
        o = 0
        for j in range(GPC):
            self.offs[j] = o
            o += self.slot_tiles[j] * TILE
        self.used_slots = o // TILE                 # non-junk tiles
        # tile -> local graph slot (-1 junk)
        t2g = np.full(self.T, -1, np.int64)
        for j in range(GPC):
            t0 = self.offs[j] // TILE
            t2g[t0:t0 + self.slot_tiles[j]] = j
        self.t2g = t2g

    def key(self):
        return (self.NS, tuple(self.slot_tiles.tolist()))


# ---------------------------------------------------------------------------
# program builder (baked on layout.key())

_PROG_CACHE = {}


def _build_program(lay: Layout):
    import concourse.bass as bass
    import concourse.tile as tile
    from concourse import bacc, mybir

    F32 = mybir.dt.float32
    F32R = mybir.dt.float32r
    AF = mybir.ActivationFunctionType
    OP = mybir.AluOpType
    NSL, T, ST = lay.NS, lay.T, lay.ST
    t2g = lay.t2g

    nc = bacc.Bacc("TRN2", target_bir_lowering=False, debug=False)

    xt_d = nc.dram_tensor("xt", [128, ST, 2, SUB, 128], F32R, kind="ExternalInput").ap()
    w_d = nc.dram_tensor("w", [128, 2, JC], F32R, kind="ExternalInput").ap()
    segw_d = nc.dram_tensor("segw", [128, T, BCHUNK], F32R, kind="ExternalInput").ap()
    maskt_d = nc.dram_tensor("maskt", [128, GPC, A], F32, kind="ExternalInput").ap()
    rsd_d = nc.dram_tensor("rsd", [(GPC + BCHUNK - 1) // BCHUNK, BCHUNK * A], F32).ap()

    pna_d = nc.dram_tensor("pna", [T, 128, A], F32, kind="ExternalOutput").ap()
    pa_d = nc.dram_tensor("pa", [GPC, A], F32, kind="ExternalOutput").ap()
    h2_d = nc.dram_tensor("h2", [GPC, A], F32, kind="ExternalOutput").ap()
    s2_d = nc.dram_tensor("s2", [GPC, A], F32, kind="ExternalOutput").ap()
    qa_d = nc.dram_tensor("qa", [GPC, A], F32, kind="ExternalOutput").ap()
    sn_d = nc.dram_tensor("sn", [GPC, A], F32, kind="ExternalOutput").ap()

    with tile.TileContext(nc) as tc:
        from contextlib import ExitStack
        with ExitStack() as ctx:
            const = ctx.enter_context(tc.tile_pool(name="const", bufs=1))
            xtp = ctx.enter_context(tc.tile_pool(name="xtp", bufs=6))
            projp = ctx.enter_context(tc.tile_pool(name="projp", bufs=2, space="PSUM"))
            segp = ctx.enter_context(tc.tile_pool(name="segp", bufs=2, space="PSUM"))
            sp = ctx.enter_context(tc.tile_pool(name="sp", bufs=4))
            smallp = ctx.enter_context(tc.tile_pool(name="smallp", bufs=6))
            rsbp = ctx.enter_context(tc.tile_pool(name="rsbp", bufs=2))
            outp = ctx.enter_context(tc.tile_pool(name="outp", bufs=4))
            bwork = ctx.enter_context(tc.tile_pool(name="bwork", bufs=2))

            # ---- constants / inputs resident in SBUF ----
            wsb = const.tile([128, 2, JC], F32R)
            nc.sync.dma_start(wsb[:], w_d)
            segwsb = const.tile([128, T, BCHUNK], F32R)
            nc.sync.dma_start(segwsb[:], segw_d)
            masksb = const.tile([128, GPC, A], F32)
            nc.sync.dma_start(masksb[:], maskt_d)
            # e_nga/e_agn cache for the whole core
            cache = const.tile([128, T, 130], F32)
            seg_slots = {}
            rsb_tiles = {}
            tbl_tiles = {}
            tblp = ctx.enter_context(tc.tile_pool(name="tblp", bufs=3))

            # ---------------- phase A ----------------
            def emit_A(st):
                xtt = xtp.tile([128, 2, SUB, 128], F32R, tag="xt")
                nc.sync.dma_start(xtt[:], xt_d[:, st, :, :, :])
                pj = projp.tile([128, SUB, 512], F32, tag="proj")
                for q in range(SUB):
                    for k in range(2):
                        nc.tensor.matmul(pj[:, q, 0:JC], xtt[:, k, q, :],
                                         wsb[:, k, :], start=(k == 0), stop=(k == 1))
                # exp of [agn|nga|node|pad] region into cache (one op)
                nc.scalar.activation(cache[:, st * SUB:(st + 1) * SUB, :],
                                     pj[:, :, 0:130], AF.Exp)
                s_t = sp.tile([128, SUB, SCOLS], F32R, tag="stage")
                # bulk-copy qna|qan into staging: S cols 256:384
                nc.scalar.copy(s_t[:, :, 256:384], pj[:, :, 130:258])
                nc.scalar.copy(s_t[:, :, 192:256],
                               cache[:, st * SUB:(st + 1) * SUB, 64:128])
                em = smallp.tile([128, SUB, A], F32, tag="em")
                for q in range(SUB):
                    t = st * SUB + q
                    gm = max(int(t2g[t]), 0)
                    nc.vector.tensor_mul(em[:, q, :], cache[:, t, 0:64],
                                         masksb[:, gm, :])
                den = smallp.tile([128, SUB], F32, tag="den")
                nc.vector.reduce_sum(den[:], em[:], axis=mybir.AxisListType.X)
                recip = smallp.tile([128, SUB], F32, tag="recip")
                nc.vector.reciprocal(recip[:], den[:])
                u = smallp.tile([128, SUB], F32, tag="u")
                nc.vector.tensor_mul(u[:], cache[:, st * SUB:(st + 1) * SUB, 128],
                                     recip[:])
                # contrib = em * u, one op (u broadcast along innermost free dim)
                ub = bass.AP(tensor=u.tensor, offset=u.offset,
                             ap=[u.ap[0], u.ap[1], [0, A]])
                nc.vector.tensor_mul(s_t[:, :, 0:64], em[:], ub)
                # t_EL = e_nga * nga_logit ; t_EQ = e_nga * qna (SBUF)
                nc.vector.tensor_mul(
                    s_t[:, :, 64:128],
                    cache[:, st * SUB:(st + 1) * SUB, 64:128], pj[:, :, 64:128])
                nc.vector.tensor_mul(
                    s_t[:, :, 128:192],
                    cache[:, st * SUB:(st + 1) * SUB, 64:128],
                    s_t[:, :, 256:320].bitcast(F32))
                # seg matmuls (one [BCHUNK,320] psum slot per graph batch)
                for q in range(SUB):
                    t = st * SUB + q
                    g = int(t2g[t])
                    if g < 0:
                        continue
                    b = g // BCHUNK
                    bt0 = lay.offs[b * BCHUNK] // TILE
                    glast = min(b * BCHUNK + BCHUNK, GPC) - 1
                    bt1 = lay.offs[glast] // TILE + lay.slot_tiles[glast]
                    if t == bt0:
                        seg_slots[b] = segp.tile([BCHUNK, 512], F32, tag="seg",
                                                 name=f"segslot{b}")
                    nc.tensor.matmul(seg_slots[b][:, 0:SCOLS], segwsb[:, t, :],
                                     s_t[:, q, :], start=(t == bt0),
                                     stop=(t == bt1 - 1))
                    if t == bt1 - 1:
                        tbl_tiles[b] = tblp.tile([BCHUNK, SCOLS], F32, tag="tbl",
                                                 name=f"tbl{b}")
                        nc.scalar.copy(tbl_tiles[b][:],
                                       seg_slots[b][:, 0:SCOLS])

            # ---------------- phase B (batch of graphs) ----------------
            def emit_B(g0, g1):
                n = g1 - g0
                tbl = tbl_tiles[g0 // BCHUNK]
                rs_t = bwork.tile([BCHUNK, A], F32, tag="rs")
                b = g0 // BCHUNK
                nc.vector.reciprocal(rs_t[:n, :], tbl[:n, 192:256])
                nc.sync.dma_start(rsd_d[b:b + 1, 0:n * A], rs_t[:n, :])
                rsb_tiles[b] = rsbp.tile([128, BCHUNK * A], F32, tag="rsb",
                                         name=f"rsb{b}")
                row = rsd_d[b:b + 1, 0:n * A]
                bcast = bass.AP(tensor=row.tensor, offset=row.offset,
                                ap=[[0, 128]] + list(row.ap)[1:])
                nc.sync.dma_start(rsb_tiles[b][:, 0:n * A], bcast)
                # raw per-graph blocks out; host finalizes H2/S2/p_a/q_a
                nc.sync.dma_start(pa_d[g0:g1, :], tbl[:n, 0:64])
                nc.sync.dma_start(h2_d[g0:g1, :], tbl[:n, 64:128])
                nc.sync.dma_start(s2_d[g0:g1, :], tbl[:n, 128:192])
                nc.sync.dma_start(sn_d[g0:g1, :], tbl[:n, 192:256])
                nc.sync.dma_start(qa_d[g0:g1, :], tbl[:n, 320:384])

            # ---------------- phase C (per graph) ----------------
            def emit_C(g):
                nt = int(lay.slot_tiles[g])
                if nt == 0:
                    return
                t0 = lay.offs[g] // TILE
                rsb = rsb_tiles[g // BCHUNK]
                j = g % BCHUNK
                out_t = outp.tile([128, nt, A], F32, tag="pnaout")
                base = rsb[:, j * A:(j + 1) * A]
                in2 = bass.AP(tensor=base.tensor, offset=base.offset,
                              ap=[base.ap[0], [0, nt], base.ap[1]])
                nc.vector.tensor_mul(out_t[:, 0:nt, :],
                                     cache[:, t0:t0 + nt, 64:128], in2)
                nc.sync.dma_start(pna_d[t0:t0 + nt, :, :].rearrange("t p a -> p t a"),
                                  out_t[:, 0:nt, :])

            # ---------------- emission ----------------
            import concourse.mybir as mybir  # noqa: F811
            graph_last_st = {}
            for g in range(GPC):
                if lay.slot_tiles[g] > 0:
                    tlast = lay.offs[g] // TILE + lay.slot_tiles[g] - 1
                    graph_last_st[g] = tlast // SUB
            done_b = 0
            pending_c = []
            for st in range(ST):
                emit_A(st)
                # B for any batch fully closed
                while done_b + BCHUNK <= GPC and all(
                        graph_last_st.get(g, -1) <= st
                        for g in range(done_b, done_b + BCHUNK)):
                    g0 = done_b
                    emit_B(g0, g0 + BCHUNK)
                    for g in range(g0, g0 + BCHUNK):
                        emit_C(g)
                    done_b += BCHUNK
            while done_b < GPC:
                g0 = done_b
                emit_B(g0, min(g0 + BCHUNK, GPC))
                for g in range(g0, min(g0 + BCHUNK, GPC)):
                    emit_C(g)
                done_b += BCHUNK

    nc.compile()
    return nc


# ---------------------------------------------------------------------------
# host marshaling + execution

def _prep_core_inputs(lay: Layout, c, values, action_mask, b_agn, W_all, segw_all):
    NSL, T, ST = lay.NS, lay.T, lay.ST
    xt = np.zeros((NSL, D), np.float32)
    for j in range(GPC):
        g = GPC * c + j
        n0, n1 = lay.gstart[g], lay.gstart[g + 1]
        o = lay.offs[j]
        xt[o:o + (n1 - n0)] = values[n0:n1]
    # [p, st, k, q, n] partition-major
    xtT = np.ascontiguousarray(
        xt.reshape(ST, SUB, 128, 2, 128).transpose(4, 0, 3, 1, 2))
    maskflat = (action_mask[GPC * c:GPC * (c + 1)].astype(np.float32)
                * np.exp(b_agn)[None, :]).reshape(-1)
    maskt = np.broadcast_to(maskflat[None, :], (128, GPC * A)).reshape(
        128, GPC, A).copy()
    return {"xt": xtT, "w": W_all, "segw": segw_all, "maskt": maskt}


def kernel(values, indices, a_action, a_node, action_mask, n_nodes,
           w_node, W_agn, b_agn, W_nga, b_nga, W_qna, b_qna, W_qan, b_qan):
    _register_ntff_hook()
    from concourse.bass_utils import run_bass_kernel_spmd

    values = np.asarray(values, np.float32)
    indices = np.asarray(indices, np.int64)
    a_action = np.asarray(a_action, np.int64)
    a_node = np.asarray(a_node, np.int64)
    action_mask = np.asarray(action_mask, bool)
    n_nodes = np.asarray(n_nodes, np.int64)
    w_node = np.asarray(w_node, np.float32)
    b_agn = np.asarray(b_agn, np.float32)
    b_nga = np.asarray(b_nga, np.float32)
    b_qna = np.asarray(b_qna, np.float32)
    b_qan = np.asarray(b_qan, np.float32)

    lay = Layout(indices)
    key = lay.key()
    if key not in _PROG_CACHE:
        _PROG_CACHE[key] = _build_program(lay)
    nc = _PROG_CACHE[key]

    # weights: cols [agn | nga | qna | qan | node | 0]
    W_all = np.zeros((D, JC), np.float32)
    W_all[:, 0:64] = np.asarray(W_agn, np.float32).T
    W_all[:, 64:128] = np.asarray(W_nga, np.float32).T
    W_all[:, 128] = w_node
    W_all[:, 130:194] = np.asarray(W_qna, np.float32).T
    W_all[:, 194:258] = np.asarray(W_qan, np.float32).T
    W_all = np.ascontiguousarray(W_all.reshape(2, 128, JC).transpose(1, 0, 2))

    # per-core segw is CORE-SPECIFIC (valid counts per graph differ)
    counts = lay.counts.reshape(NCORES, GPC)
    in_maps = []
    for c in range(NCORES):
        segw = np.zeros((lay.T, 128, BCHUNK), np.float32)
        for j in range(GPC):
            t0 = lay.offs[j] // TILE
            nv = int(counts[c, j])
            col = j % BCHUNK
            full, rem = nv // TILE, nv % TILE
            segw[t0:t0 + full, :, col] = 1.0
            if rem:
                segw[t0 + full, 0:rem, col] = 1.0
        segw = np.ascontiguousarray(segw.transpose(1, 0, 2))
        in_maps.append(_prep_core_inputs(lay, c, values, action_mask, b_agn,
                                         W_all, segw))

    import os
    if os.environ.get("KSIM") == "1":
        from concourse.bass_interp import CoreSim
        sim = CoreSim(nc)
        for name, arr in in_maps[0].items():
            sim.tensor(name)[:] = arr
        sim.simulate()
        r0 = {name: np.array(sim.tensor(name))
              for name in ["pna", "pa", "h2", "s2", "qa", "sn"]}
        zeros = {k: np.zeros_like(v) for k, v in r0.items()}

        class _R:
            results = [r0] + [zeros] * (NCORES - 1)
            exec_time_ns = None
        res = _R()
    else:
        res = run_bass_kernel_spmd(nc, in_maps, list(range(NCORES)))
    kernel.last_results = res

    # ---------------- host assembly ----------------
    pa_raw = np.zeros((G, A), np.float32)
    segEL = np.zeros((G, A), np.float32)
    segEQ = np.zeros((G, A), np.float32)
    s_nga = np.zeros((G, A), np.float32)
    q_a = np.zeros((G, A), np.float32)
    p_n__a = np.zeros((N, A), np.float32)
    for c in range(NCORES):
        r = res.results[c]
        pa_raw[GPC * c:GPC * (c + 1)] = r["pa"]
        segEL[GPC * c:GPC * (c + 1)] = r["h2"]
        segEQ[GPC * c:GPC * (c + 1)] = r["s2"]
        s_nga[GPC * c:GPC * (c + 1)] = r["sn"]
        q_a[GPC * c:GPC * (c + 1)] = r["qa"]
        pna = r["pna"].reshape(lay.NS, A)
        for j in range(GPC):
            g = GPC * c + j
            n0, n1 = lay.gstart[g], lay.gstart[g + 1]
            o = lay.offs[j]
            p_n__a[n0:n1] = pna[o:o + (n1 - n0)]

    # per-graph [G, A] finalization on host (tiny)
    rs = 1.0 / s_nga
    p_a = pa_raw / pa_raw.sum(axis=-1, keepdims=True)
    H2 = np.log(s_nga) - rs * segEL
    S2 = rs * segEQ
    # bias corrections (biases are zero in the reference setup, but be exact)
    ng = n_nodes.astype(np.float32)
    q_a = q_a + ng[:, None] * b_qan[None, :]
    S2 = S2 + b_qna[None, :]

    # entropy
    H_a = -np.sum(p_a * np.log(p_a + EPS), axis=-1)
    entropy = H_a + np.sum(p_a * H2, axis=-1)
    # value
    value = np.sum(p_a * (q_a + S2), axis=-1)
    # logprob
    g_ar = np.arange(G)
    pa_sel = p_a[g_ar, a_action]
    gprime = indices[a_node]                      # graph of the selected node
    mask_sel = action_mask[gprime, a_action]
    pnam_sel = np.where(mask_sel,
                        p_n__a[a_node, a_action],
                        1.0 / n_nodes[gprime].astype(np.float32))
    logprob = np.log(pa_sel + EPS) + np.log(pnam_sel + EPS)

    return (logprob.astype(np.float32), entropy.astype(np.float32),
            value.astype(np.float32), p_a, p_n__a)


# revision 40
# speedup vs baseline: 1.1172x; 1.0133x over previous
"""Trainium2 Bass kernel for nn_ActionThenNodePolicy (segment softmax policy head).

kernel(**inputs) takes FULL inputs (as from setup_inputs()) and returns the full
(logprob, entropy, value, p_a, p_n__a) tuple, computing the heavy O(N*D)/O(N*A)
work on 8 NeuronCores via bass/Tile.

Self-contained: hardcodes N=131072, G=256, D=256, A=64, 8 cores.
"""
import sys
import types
import numpy as np

# ---------------------------------------------------------------------------
# constants
N, G, D, A = 131072, 256, 256, 64
NCORES = 8
GPC = G // NCORES          # graphs per core
NEG = -1e9
EPS = 1e-10
TILE = 128                 # nodes per tile
SUB = 3                    # tiles per supertile (psum-bank limited)
STN = TILE * SUB           # nodes per supertile
JC = 258                   # proj output cols: agn(64) nga(64) qna(64) qan(64) node(1) pad(1)
SCOLS = 384                # contrib(64) tEL(64) tEQ(64) e_nga(64) qna(64) qan(64)
BCHUNK = 8                 # graphs per phase-B batch (also seg psum slot rows)


def _register_ntff_hook():
    try:
        from trn_agent_boot.trn_boot import _ntff_profile_via_ctypes
        if 'antenv.axon_hooks' not in sys.modules:
            hook = _ntff_profile_via_ctypes('/opt/axon/libaxon_pjrt.so')
            mod = types.ModuleType('antenv.axon_hooks')
            mod.get_axon_ntff_profile_hook = lambda: hook
            sys.modules['antenv.axon_hooks'] = mod
    except Exception:
        pass


# ---------------------------------------------------------------------------
# layout computed from runtime `indices` (sorted graph ids)

class Layout:
    def __init__(self, indices):
        counts = np.bincount(np.asarray(indices, np.int64), minlength=G)
        self.counts = counts                       # n_g actual per graph
        Lg = ((counts + TILE - 1) // TILE) * TILE  # padded length per graph
        self.Lg = Lg
        per_core = Lg.reshape(NCORES, GPC).sum(1)
        ns = int(per_core.max())
        ns = ((ns + STN - 1) // STN) * STN
        self.NS = ns
        self.T = ns // TILE
        self.ST = ns // STN
        # per-core per-graph offsets in padded space
        self.offs = np.zeros((NCORES, GPC), np.int64)
        for c in range(NCORES):
            o = 0
            for j in range(GPC):
                self.offs[c, j] = o
                o += Lg[GPC * c + j]
        self.used = per_core                        # padded nodes actually used per core
        # graph start offsets in the ORIGINAL node array
        self.gstart = np.zeros(G + 1, np.int64)
        self.gstart[1:] = np.cumsum(counts)
        # per-core tile -> local graph (or -1 for junk tail tiles); identical
        # tile->graph maps are required across cores for a single SPMD program,
        # so the program bakes PER-CORE structure only if maps match; otherwise
        # we bake the union (core-specific seg matmul emission is impossible in
        # SPMD).  We therefore make the PROGRAM data-driven per tile only via
        # input tensors (segw), but seg-matmul grouping (which psum slot, which
        # start/stop) must be identical across cores -> we pad every core to
        # the SAME per-graph tile counts: use max tiles per graph-slot across
        # cores.
        TgSlot = (Lg.reshape(NCORES, GPC) // TILE)
        self.slot_tiles = TgSlot.max(0)             # tiles per local-graph slot, shared
        ns2 = int(self.slot_tiles.sum()) * TILE
        ns2 = ((ns2 + STN - 1) // STN) * STN
        self.NS = ns2
        self.T = ns2 // TILE
        self.ST = ns2 // STN
        self.offs = np.zeros(GPC, np.int64)
        o = 0
        for j in range(GPC):
            self.offs[j] = o
            o += self.slot_tiles[j] * TILE
        self.used_slots = o // TILE                 # non-junk tiles
        # tile -> local graph slot (-1 junk)
        t2g = np.full(self.T, -1, np.int64)
        for j in range(GPC):
            t0 = self.offs[j] // TILE
            t2g[t0:t0 + self.slot_tiles[j]] = j
        self.t2g = t2g

    def key(self):
        return (self.NS, tuple(self.slot_tiles.tolist()))


# ---------------------------------------------------------------------------
# program builder (baked on layout.key())

_PROG_CACHE = {}


def _build_program(lay: Layout):
    import concourse.bass as bass
    import concourse.tile as tile
    from concourse import bacc, mybir

    F32 = mybir.dt.float32
    F32R = mybir.dt.float32r
    AF = mybir.ActivationFunctionType
    OP = mybir.AluOpType
    NSL, T, ST = lay.NS, lay.T, lay.ST
    t2g = lay.t2g

    nc = bacc.Bacc("TRN2", target_bir_lowering=False, debug=False)

    xt_d = nc.dram_tensor("xt", [128, ST, 2, SUB, 128], F32R, kind="ExternalInput").ap()
    w_d = nc.dram_tensor("w", [128, 2, JC], F32R, kind="ExternalInput").ap()
    segw_d = nc.dram_tensor("segw", [128, T, BCHUNK], F32R, kind="ExternalInput").ap()
    maskt_d = nc.dram_tensor("maskt", [128, GPC, A], F32, kind="ExternalInput").ap()
    rsd_d = nc.dram_tensor("rsd", [(GPC + BCHUNK - 1) // BCHUNK, BCHUNK * A], F32).ap()

    pna_d = nc.dram_tensor("pna", [T, 128, A], F32, kind="ExternalOutput").ap()
    pa_d = nc.dram_tensor("pa", [GPC, A], F32, kind="ExternalOutput").ap()
    h2_d = nc.dram_tensor("h2", [GPC, A], F32, kind="ExternalOutput").ap()
    s2_d = nc.dram_tensor("s2", [GPC, A], F32, kind="ExternalOutput").ap()
    qa_d = nc.dram_tensor("qa", [GPC, A], F32, kind="ExternalOutput").ap()
    sn_d = nc.dram_tensor("sn", [GPC, A], F32, kind="ExternalOutput").ap()

    with tile.TileContext(nc) as tc:
        from contextlib import ExitStack
        with ExitStack() as ctx:
            const = ctx.enter_context(tc.tile_pool(name="const", bufs=1))
            xtp = ctx.enter_context(tc.tile_pool(name="xtp", bufs=6))
            projp = ctx.enter_context(tc.tile_pool(name="projp", bufs=2, space="PSUM"))
            segp = ctx.enter_context(tc.tile_pool(name="segp", bufs=2, space="PSUM"))
            sp = ctx.enter_context(tc.tile_pool(name="sp", bufs=4))
            smallp = ctx.enter_context(tc.tile_pool(name="smallp", bufs=6))
            rsbp = ctx.enter_context(tc.tile_pool(name="rsbp", bufs=2))
            outp = ctx.enter_context(tc.tile_pool(name="outp", bufs=4))
            bwork = ctx.enter_context(tc.tile_pool(name="bwork", bufs=2))

            # ---- constants / inputs resident in SBUF ----
            wsb = const.tile([128, 2, JC], F32R)
            nc.sync.dma_start(wsb[:], w_d)
            segwsb = const.tile([128, T, BCHUNK], F32R)
            nc.sync.dma_start(segwsb[:], segw_d)
            masksb = const.tile([128, GPC, A], F32)
            nc.sync.dma_start(masksb[:], maskt_d)
            # e_nga/e_agn cache for the whole core
            cache = const.tile([128, T, 130], F32)
            seg_slots = {}
            rsb_tiles = {}
            tbl_tiles = {}
            tblp = ctx.enter_context(tc.tile_pool(name="tblp", bufs=3))

            # ---------------- phase A ----------------
            def emit_A(st):
                xtt = xtp.tile([128, 2, SUB, 128], F32R, tag="xt")
                nc.sync.dma_start(xtt[:], xt_d[:, st, :, :, :])
                pj = projp.tile([128, SUB, 512], F32, tag="proj")
                for q in range(SUB):
                    for k in range(2):
                        nc.tensor.matmul(pj[:, q, 0:JC], xtt[:, k, q, :],
                                         wsb[:, k, :], start=(k == 0), stop=(k == 1))
                # exp of [agn|nga|node|pad] region into cache (one op)
                nc.scalar.activation(cache[:, st * SUB:(st + 1) * SUB, :],
                                     pj[:, :, 0:130], AF.Exp)
                s_t = sp.tile([128, SUB, SCOLS], F32R, tag="stage")
                # bulk-copy qna|qan into staging: S cols 256:384
                nc.scalar.copy(s_t[:, :, 256:384], pj[:, :, 130:258])
                nc.scalar.copy(s_t[:, :, 192:256],
                               cache[:, st * SUB:(st + 1) * SUB, 64:128])
                em = smallp.tile([128, SUB, A], F32, tag="em")
                for q in range(SUB):
                    t = st * SUB + q
                    gm = max(int(t2g[t]), 0)
                    nc.vector.tensor_mul(em[:, q, :], cache[:, t, 0:64],
                                         masksb[:, gm, :])
                den = smallp.tile([128, SUB], F32, tag="den")
                nc.vector.reduce_sum(den[:], em[:], axis=mybir.AxisListType.X)
                recip = smallp.tile([128, SUB], F32, tag="recip")
                nc.vector.reciprocal(recip[:], den[:])
                u = smallp.tile([128, SUB], F32, tag="u")
                nc.vector.tensor_mul(u[:], cache[:, st * SUB:(st + 1) * SUB, 128],
                                     recip[:])
                # contrib = em * u, one op (u broadcast along innermost free dim)
                ub = bass.AP(tensor=u.tensor, offset=u.offset,
                             ap=[u.ap[0], u.ap[1], [0, A]])
                nc.vector.tensor_mul(s_t[:, :, 0:64], em[:], ub)
                # t_EL = e_nga * nga_logit ; t_EQ = e_nga * qna (SBUF)
                nc.vector.tensor_mul(
                    s_t[:, :, 64:128],
                    cache[:, st * SUB:(st + 1) * SUB, 64:128], pj[:, :, 64:128])
                nc.vector.tensor_mul(
                    s_t[:, :, 128:192],
                    cache[:, st * SUB:(st + 1) * SUB, 64:128],
                    s_t[:, :, 256:320].bitcast(F32))
                # seg matmuls (one [BCHUNK,320] psum slot per graph batch)
                for q in range(SUB):
                    t = st * SUB + q
                    g = int(t2g[t])
                    if g < 0:
                        continue
                    b = g // BCHUNK
                    bt0 = lay.offs[b * BCHUNK] // TILE
                    glast = min(b * BCHUNK + BCHUNK, GPC) - 1
                    bt1 = lay.offs[glast] // TILE + lay.slot_tiles[glast]
                    if t == bt0:
                        seg_slots[b] = segp.tile([BCHUNK, 512], F32, tag="seg",
                                                 name=f"segslot{b}")
                    nc.tensor.matmul(seg_slots[b][:, 0:SCOLS], segwsb[:, t, :],
                                     s_t[:, q, :], start=(t == bt0),
                                     stop=(t == bt1 - 1))
                    if t == bt1 - 1:
                        tbl_tiles[b] = tblp.tile([BCHUNK, SCOLS], F32, tag="tbl",
                                                 name=f"tbl{b}")
                        nc.scalar.copy(tbl_tiles[b][:],
                                       seg_slots[b][:, 0:SCOLS])

            # ---------------- phase B (batch of graphs) ----------------
            def emit_B(g0, g1):
                n = g1 - g0
                tbl = tbl_tiles[g0 // BCHUNK]
                rs_t = bwork.tile([BCHUNK, A], F32, tag="rs")
                b = g0 // BCHUNK
                nc.vector.reciprocal(rs_t[:n, :], tbl[:n, 192:256])
                nc.sync.dma_start(rsd_d[b:b + 1, 0:n * A], rs_t[:n, :])
                rsb_tiles[b] = rsbp.tile([128, BCHUNK * A], F32, tag="rsb",
                                         name=f"rsb{b}")
                row = rsd_d[b:b + 1, 0:n * A]
                bcast = bass.AP(tensor=row.tensor, offset=row.offset,
                                ap=[[0, 128]] + list(row.ap)[1:])
                nc.sync.dma_start(rsb_tiles[b][:, 0:n * A], bcast)
                # raw per-graph blocks out; host finalizes H2/S2/p_a/q_a
                nc.sync.dma_start(pa_d[g0:g1, :], tbl[:n, 0:64])
                nc.sync.dma_start(h2_d[g0:g1, :], tbl[:n, 64:128])
                nc.sync.dma_start(s2_d[g0:g1, :], tbl[:n, 128:192])
                nc.sync.dma_start(sn_d[g0:g1, :], tbl[:n, 192:256])
                nc.sync.dma_start(qa_d[g0:g1, :], tbl[:n, 320:384])

            # ---------------- phase C (per graph) ----------------
            def emit_C(g):
                nt = int(lay.slot_tiles[g])
                if nt == 0:
                    return
                t0 = lay.offs[g] // TILE
                rsb = rsb_tiles[g // BCHUNK]
                j = g % BCHUNK
                out_t = outp.tile([128, nt, A], F32, tag="pnaout")
                base = rsb[:, j * A:(j + 1) * A]
                in2 = bass.AP(tensor=base.tensor, offset=base.offset,
                              ap=[base.ap[0], [0, nt], base.ap[1]])
                nc.vector.tensor_mul(out_t[:, 0:nt, :],
                                     cache[:, t0:t0 + nt, 64:128], in2)
                nc.sync.dma_start(pna_d[t0:t0 + nt, :, :].rearrange("t p a -> p t a"),
                                  out_t[:, 0:nt, :])

            # ---------------- emission ----------------
            import concourse.mybir as mybir  # noqa: F811
            graph_last_st = {}
            for g in range(GPC):
                if lay.slot_tiles[g] > 0:
                    tlast = lay.offs[g] // TILE + lay.slot_tiles[g] - 1
                    graph_last_st[g] = tlast // SUB
            done_b = 0
            pending_c = []
            for st in range(ST):
                emit_A(st)
                # B for any batch fully closed
                while done_b + BCHUNK <= GPC and all(
                        graph_last_st.get(g, -1) <= st
                        for g in range(done_b, done_b + BCHUNK)):
                    g0 = done_b
                    emit_B(g0, g0 + BCHUNK)
                    for g in range(g0, g0 + BCHUNK):
                        emit_C(g)
                    done_b += BCHUNK
            while done_b < GPC:
                g0 = done_b
                emit_B(g0, min(g0 + BCHUNK, GPC))
                for g in range(g0, min(g0 + BCHUNK, GPC)):
                    emit_C(g)
                done_b += BCHUNK

    nc.compile()
    return nc


# ---------------------------------------------------------------------------
# host marshaling + execution

def _prep_core_inputs(lay: Layout, c, values, action_mask, b_agn, W_all, segw_all):
    NSL, T, ST = lay.NS, lay.T, lay.ST
    xt = np.zeros((NSL, D), np.float32)
    for j in range(GPC):
        g = GPC * c + j
        n0, n1 = lay.gstart[g], lay.gstart[g + 1]
        o = lay.offs[j]
        xt[o:o + (n1 - n0)] = values[n0:n1]
    # [p, st, k, q, n] partition-major
    xtT = np.ascontiguousarray(
        xt.reshape(ST, SUB, 128, 2, 128).transpose(4, 0, 3, 1, 2))
    maskflat = (action_mask[GPC * c:GPC * (c + 1)].astype(np.float32)
                * np.exp(b_agn)[None, :]).reshape(-1)
    maskt = np.broadcast_to(maskflat[None, :], (128, GPC * A)).reshape(
        128, GPC, A).copy()
    return {"xt": xtT, "w": W_all, "segw": segw_all, "maskt": maskt}


def kernel(values, indices, a_action, a_node, action_mask, n_nodes,
           w_node, W_agn, b_agn, W_nga, b_nga, W_qna, b_qna, W_qan, b_qan):
    _register_ntff_hook()
    from concourse.bass_utils import run_bass_kernel_spmd

    values = np.asarray(values, np.float32)
    indices = np.asarray(indices, np.int64)
    a_action = np.asarray(a_action, np.int64)
    a_node = np.asarray(a_node, np.int64)
    action_mask = np.asarray(action_mask, bool)
    n_nodes = np.asarray(n_nodes, np.int64)
    w_node = np.asarray(w_node, np.float32)
    b_agn = np.asarray(b_agn, np.float32)
    b_nga = np.asarray(b_nga, np.float32)
    b_qna = np.asarray(b_qna, np.float32)
    b_qan = np.asarray(b_qan, np.float32)

    lay = Layout(indices)
    key = lay.key()
    if key not in _PROG_CACHE:
        _PROG_CACHE[key] = _build_program(lay)
    nc = _PROG_CACHE[key]

    # weights: cols [agn | nga | qna | qan | node | 0]
    W_all = np.zeros((D, JC), np.float32)
    W_all[:, 0:64] = np.asarray(W_agn, np.float32).T
    W_all[:, 64:128] = np.asarray(W_nga, np.float32).T
    W_all[:, 128] = w_node
    W_all[:, 130:194] = np.asarray(W_qna, np.float32).T
    W_all[:, 194:258] = np.asarray(W_qan, np.float32).T
    W_all = np.ascontiguousarray(W_all.reshape(2, 128, JC).transpose(1, 0, 2))

    # per-core segw is CORE-SPECIFIC (valid counts per graph differ)
    counts = lay.counts.reshape(NCORES, GPC)
    in_maps = []
    for c in range(NCORES):
        segw = np.zeros((lay.T, 128, BCHUNK), np.float32)
        for j in range(GPC):
            t0 = lay.offs[j] // TILE
            nv = int(counts[c, j])
            col = j % BCHUNK
            full, rem = nv // TILE, nv % TILE
            segw[t0:t0 + full, :, col] = 1.0
            if rem:
                segw[t0 + full, 0:rem, col] = 1.0
        segw = np.ascontiguousarray(segw.transpose(1, 0, 2))
        in_maps.append(_prep_core_inputs(lay, c, values, action_mask, b_agn,
                                         W_all, segw))

    import os
    if os.environ.get("KSIM") == "1":
        from concourse.bass_interp import CoreSim
        sim = CoreSim(nc)
        for name, arr in in_maps[0].items():
            sim.tensor(name)[:] = arr
        sim.simulate()
        r0 = {name: np.array(sim.tensor(name))
              for name in ["pna", "pa", "h2", "s2", "qa", "sn"]}
        zeros = {k: np.zeros_like(v) for k, v in r0.items()}

        class _R:
            results = [r0] + [zeros] * (NCORES - 1)
            exec_time_ns = None
        res = _R()
    else:
        res = run_bass_kernel_spmd(nc, in_maps, list(range(NCORES)))
    kernel.last_results = res

    # ---------------- host assembly ----------------
    pa_raw = np.zeros((G, A), np.float32)
    segEL = np.zeros((G, A), np.float32)
    segEQ = np.zeros((G, A), np.float32)
    s_nga = np.zeros((G, A), np.float32)
    q_a = np.zeros((G, A), np.float32)
    p_n__a = np.zeros((N, A), np.float32)
    for c in range(NCORES):
        r = res.results[c]
        pa_raw[GPC * c:GPC * (c + 1)] = r["pa"]
        segEL[GPC * c:GPC * (c + 1)] = r["h2"]
        segEQ[GPC * c:GPC * (c + 1)] = r["s2"]
        s_nga[GPC * c:GPC * (c + 1)] = r["sn"]
        q_a[GPC * c:GPC * (c + 1)] = r["qa"]
        pna = r["pna"].reshape(lay.NS, A)
        for j in range(GPC):
            g = GPC * c + j
            n0, n1 = lay.gstart[g], lay.gstart[g + 1]
            o = lay.offs[j]
            p_n__a[n0:n1] = pna[o:o + (n1 - n0)]

    # per-graph [G, A] finalization on host (tiny)
    rs = 1.0 / s_nga
    p_a = pa_raw / pa_raw.sum(axis=-1, keepdims=True)
    H2 = np.log(s_nga) - rs * segEL
    S2 = rs * segEQ
    # bias corrections (biases are zero in the reference setup, but be exact)
    ng = n_nodes.astype(np.float32)
    q_a = q_a + ng[:, None] * b_qan[None, :]
    S2 = S2 + b_qna[None, :]

    # entropy
    H_a = -np.sum(p_a * np.log(p_a + EPS), axis=-1)
    entropy = H_a + np.sum(p_a * H2, axis=-1)
    # value
    value = np.sum(p_a * (q_a + S2), axis=-1)
    # logprob
    g_ar = np.arange(G)
    pa_sel = p_a[g_ar, a_action]
    gprime = indices[a_node]                      # graph of the selected node
    mask_sel = action_mask[gprime, a_action]
    pnam_sel = np.where(mask_sel,
                        p_n__a[a_node, a_action],
                        1.0 / n_nodes[gprime].astype(np.float32))
    logprob = np.log(pa_sel + EPS) + np.log(pnam_sel + EPS)

    return (logprob.astype(np.float32), entropy.astype(np.float32),
            value.astype(np.float32), p_a, p_n__a)
